# revision 1
# baseline (speedup 1.0000x reference)
"""Compressed Interaction Network (CIN) forward on 8 Trainium2 NeuronCores.

Math (per batch item, m=32 fields, d=64 embed, H=256 hidden):
    x0 = x[i]                          # (m, d)
    h  = x0
    layer l in 0..2:
        z = outer(x0, h) over d        # (m*n, d), z[(a,b),:] = x0[a,:]*h[b,:]
        y = relu(W_l^T z + b_l)        # (H, d)
        xcur, h = split_half(y) (layers 0,1); xcur = h = y (layer 2)
    f = concat(xcur_0, xcur_1, xcur_2) # (512, d)
    out[i] = sum_d(f) @ fc_W + fc_b    # scalar

Mapping: batch 1024 -> 8 cores x 128 items, 16 groups of 8 items per core.
 - Outer-product operands are built with DMA broadcast reads (stride-0 APs),
   one DMA per tile so consumers wait on a single DMA semaphore lane.
 - z tiles computed on VectorE in fp16 (2x mode), layout [k-part, (i, m, d)].
 - Conv matmuls on PE: stationary W chunks [128, 128] fp16, moving z
   [128, 512] (8 items x 64 d), accumulated over k-chunks in fp32 PSUM.
 - Bias+ReLU fused into the PSUM->SBUF move on ScalarE; per-item d-sums for
   the final FC are fused there too via accum_out.
 - Final dot: PE matmul of [128,1] fc weight chunks against [128, 128] sums.
"""

import numpy as np

import concourse.bass as bass
import concourse.tile as tile
from concourse import mybir
from concourse.bass_utils import run_bass_kernel_spmd

N_CORES = 8
B_TOTAL = 1024
B_CORE = B_TOTAL // N_CORES  # 128
M = 32  # num fields
D = 64  # embed dim
H = 256  # conv output channels
GROUP = 8  # items per group (512 moving columns)
N_GROUPS = B_CORE // GROUP  # 16
MD = M * D  # 2048, elements per item row

F16 = mybir.dt.float16
F32 = mybir.dt.float32
RELU = mybir.ActivationFunctionType.Relu
IDENT = mybir.ActivationFunctionType.Identity


def build():
    nc = bass.Bass()
    xh = nc.declare_dram_parameter("xh", [B_CORE, M, D], F16, isOutput=False)
    # x rows tiled 4x along the field axis: xr[i, p, d] = x[i, p % 32, d]
    xr = nc.declare_dram_parameter("xr", [B_CORE, 128, D], F16, isOutput=False)
    w0 = nc.declare_dram_parameter("w0", [8, 128, H], F16, isOutput=False)
    w1 = nc.declare_dram_parameter("w1", [32, 128, H], F16, isOutput=False)
    w2 = nc.declare_dram_parameter("w2", [32, 128, H], F16, isOutput=False)
    bia = nc.declare_dram_parameter("bia", [128, 3, 2], F32, isOutput=False)
    fcw = nc.declare_dram_parameter("fcw", [128, 4], F32, isOutput=False)
    fcb = nc.declare_dram_parameter("fcb", [1, 1], F32, isOutput=False)
    out = nc.declare_dram_parameter("out", [B_CORE, 1], F32, isOutput=True)

    with tile.TileContext(nc) as tc:
        with (
            tc.tile_pool(name="consts", bufs=1) as consts,
            tc.tile_pool(name="bpool", bufs=2) as bpool,
            tc.tile_pool(name="epool", bufs=4) as epool,
            tc.tile_pool(name="zpool", bufs=3) as zpool,
            tc.tile_pool(name="hpool", bufs=4) as hpool,
            tc.tile_pool(name="spool", bufs=1) as spool,
            tc.tile_pool(name="ppool", bufs=6, space="PSUM") as ppool,
            tc.tile_pool(name="fcp", bufs=1, space="PSUM") as fcp,
        ):
            # --- resident constants ---
            w0_sb = consts.tile([128, 8, H], F16, tag="w0")
            nc.sync.dma_start(w0_sb[:], w0[:].rearrange("c k o -> k c o"))
            w1_sb = consts.tile([128, 32, H], F16, tag="w1")
            nc.sync.dma_start(w1_sb[:], w1[:].rearrange("c k o -> k c o"))
            w2_sb = consts.tile([128, 32, H], F16, tag="w2")
            nc.sync.dma_start(w2_sb[:], w2[:].rearrange("c k o -> k c o"))
            bia_sb = consts.tile([128, 3, 2], F32, tag="bia")
            nc.sync.dma_start(bia_sb[:], bia[:])
            fcw_sb = consts.tile([128, 4], F32, tag="fcw")
            nc.sync.dma_start(fcw_sb[:], fcw[:])
            fcb_sb = consts.tile([1, 1], F32, tag="fcb")
            nc.sync.dma_start(fcb_sb[:], fcb[:])

            # per-item d-sums of the relu'd xs channels, [channel, item]
            s_tiles = [
                spool.tile([128, B_CORE], F32, tag=f"s{c}", name=f"s{c}")
                for c in range(4)
            ]

            for g in range(N_GROUPS):
                i0 = g * GROUP

                # B[p, i, m, d] = x_i[m, d] for every partition p
                # (one DMA: partition loop stride 0, (m d) merged contiguous)
                Bg = bpool.tile([128, GROUP, M, D], F16, tag="B")
                src = bass.AP(
                    tensor=xh,
                    offset=i0 * MD,
                    ap=[[0, 128], [MD, GROUP], [1, MD]],
                )
                nc.sync.dma_start(Bg[:], src)

                # R[p, i, d] = x_i[p % 32, d]  (from the host-tiled copy)
                Rg = epool.tile([128, GROUP, D], F16, tag="R")
                src = bass.AP(
                    tensor=xr,
                    offset=i0 * 128 * D,
                    ap=[[D, 128], [128 * D, GROUP], [1, D]],
                )
                nc.sync.dma_start(Rg[:], src)

                # ---------- layer 0: z0[(a,b)] = x[4c + p//32] * x[p%32] ----------
                # per 32-partition block s: z0[32s:32s+32] = R-block * B[:, :, 4c+s]
                # (operands of an engine op must share the partition range)
                ps0 = [
                    ppool.tile([128, GROUP * D], F32, tag="yps", name="ps0")
                    for _ in range(2)
                ]
                for c in range(8):
                    z0 = epool.tile([128, GROUP, D], F16, tag="z0")
                    for s in range(4):
                        pr = slice(32 * s, 32 * (s + 1))
                        nc.vector.tensor_mul(
                            z0[pr, :, :],
                            Rg[pr, :, :],
                            Bg[pr, :, 4 * c + s, :],
                        )
                    for oc in range(2):
                        nc.tensor.matmul(
                            ps0[oc][:],
                            w0_sb[:, c, oc * 128 : (oc + 1) * 128],
                            z0[:],
                            start=(c == 0),
                            stop=(c == 7),
                        )

                # psum -> sbuf with fused bias+relu; chunk1 becomes next h,
                # chunk0 only needs its per-item d-sums (accum_out into s0)
                h1 = hpool.tile([128, GROUP, D], F16, tag="h1")
                nc.scalar.activation(h1[:], ps0[1][:], RELU, bias=bia_sb[:, 0, 1:2])
                for i in range(GROUP):
                    sl = slice(i * D, (i + 1) * D)
                    nc.scalar.activation(
                        ps0[0][:, sl],
                        ps0[0][:, sl],
                        RELU,
                        bias=bia_sb[:, 0, 0:1],
                        accum_out=s_tiles[0][:, i0 + i : i0 + i + 1],
                    )

                # ---------- layers 1 and 2 ----------
                for lay in range(2):
                    w_sb = w1_sb if lay == 0 else w2_sb
                    h_in = h1 if lay == 0 else h2
                    ps = [
                        ppool.tile([128, GROUP * D], F32, tag="yps", name="ps")
                        for _ in range(2)
                    ]
                    for mb in range(8):
                        zt = zpool.tile([128, GROUP, 4, D], F16, tag="z")
                        nc.vector.tensor_mul(
                            zt[:],
                            h_in[:, :, None, :].to_broadcast((128, GROUP, 4, D)),
                            Bg[:, :, 4 * mb : 4 * mb + 4, :],
                        )
                        for mm in range(4):
                            m = 4 * mb + mm
                            for oc in range(2):
                                nc.tensor.matmul(
                                    ps[oc][:],
                                    w_sb[:, m, oc * 128 : (oc + 1) * 128],
                                    zt[:, :, mm, :],
                                    start=(m == 0),
                                    stop=(m == 31),
                                )
                    if lay == 0:
                        # split_half: chunk0 -> s1 sums, chunk1 -> h2
                        h2 = hpool.tile([128, GROUP, D], F16, tag="h2")
                        nc.scalar.activation(
                            h2[:], ps[1][:], RELU, bias=bia_sb[:, 1, 1:2]
                        )
                        for i in range(GROUP):
                            sl = slice(i * D, (i + 1) * D)
                            nc.scalar.activation(
                                ps[0][:, sl],
                                ps[0][:, sl],
                                RELU,
                                bias=bia_sb[:, 1, 0:1],
                                accum_out=s_tiles[1][:, i0 + i : i0 + i + 1],
                            )
                    else:
                        # last layer: both chunks feed the FC sums (s2, s3)
                        for oc in range(2):
                            for i in range(GROUP):
                                sl = slice(i * D, (i + 1) * D)
                                nc.scalar.activation(
                                    ps[oc][:, sl],
                                    ps[oc][:, sl],
                                    RELU,
                                    bias=bia_sb[:, 2, oc : oc + 1],
                                    accum_out=s_tiles[2 + oc][
                                        :, i0 + i : i0 + i + 1
                                    ],
                                )

            # ---------- final FC: out[i] = sum_c fcw[c] * s[c, i] + fcb ----------
            fc_ps = fcp.tile([1, B_CORE], F32, tag="fc")
            for c in range(4):
                nc.tensor.matmul(
                    fc_ps[:],
                    fcw_sb[:, c : c + 1],
                    s_tiles[c][:],
                    start=(c == 0),
                    stop=(c == 3),
                )
            osb = consts.tile([1, B_CORE], F32, tag="osb")
            nc.scalar.activation(osb[:], fc_ps[:], IDENT, bias=fcb_sb[0:1, 0:1])
            nc.sync.dma_start(out[:], osb[:])

    _legalize_waits(nc)
    return nc


def _legalize_waits(nc, max_waits=1):
    """walrus codegen allows at most 2 semaphore waits per instruction; spill
    the excess onto NoOps injected just before the offender on the same
    engine (same-engine FIFO makes this ordering-equivalent)."""
    for bb in nc.main_func.blocks:
        insts = bb.instructions
        i = 0
        new_list = []
        changed = False
        for ins in insts:
            si = ins.sync_info
            if si is not None and si.on_wait and len(si.on_wait) > max_waits:
                waits = list(si.on_wait)
                extra, keep = waits[:-max_waits], waits[-max_waits:]
                k = 0
                while k < len(extra):
                    chunk = extra[k : k + max_waits]
                    nop = mybir.InstNoOp(name=f"{ins.name}-w{k}", ins=[], outs=[])
                    nop.engine = ins.engine
                    nop.sync_info = mybir.SyncInfo(on_wait=chunk, on_update=[])
                    new_list.append(nop)
                    k += max_waits
                ins.sync_info = mybir.SyncInfo(
                    on_wait=keep,
                    on_update=list(si.on_update) if si.on_update else [],
                )
                changed = True
            new_list.append(ins)
        if changed:
            if hasattr(bb, "set_instructions"):
                bb.set_instructions(new_list)
            else:
                insts.clear()
                insts.extend(new_list)
                if len(bb.instructions) != len(new_list):
                    bb.instructions = new_list


def prep_inputs(x, W0, b0, W1, b1, W2, b2, fc_W, fc_b):
    """Host-side reshape/cast into the per-core input maps."""
    xh = np.ascontiguousarray(x.astype(np.float16))
    xr = np.ascontiguousarray(
        np.tile(xh.reshape(B_TOTAL, 1, M, D), (1, 4, 1, 1)).reshape(
            B_TOTAL, 128, D
        )
    )
    w0 = np.ascontiguousarray(W0.astype(np.float16).reshape(8, 128, H))
    w1 = np.ascontiguousarray(W1.astype(np.float16).reshape(32, 128, H))
    w2 = np.ascontiguousarray(W2.astype(np.float16).reshape(32, 128, H))
    bia = np.ascontiguousarray(
        np.stack([b0, b1, b2]).reshape(3, 2, 128).transpose(2, 0, 1).astype(np.float32)
    )
    fcw = np.ascontiguousarray(fc_W.reshape(4, 128).T.astype(np.float32))
    fcb = np.ascontiguousarray(fc_b.reshape(1, 1).astype(np.float32))
    shared = {"w0": w0, "w1": w1, "w2": w2, "bia": bia, "fcw": fcw, "fcb": fcb}
    return [
        {
            "xh": xh[i * B_CORE : (i + 1) * B_CORE],
            "xr": xr[i * B_CORE : (i + 1) * B_CORE],
            **shared,
        }
        for i in range(N_CORES)
    ]


_NC = None


def _get_nc():
    global _NC
    if _NC is None:
        _NC = build()
    return _NC


def kernel(**inputs):
    in_maps = prep_inputs(**inputs)
    res = run_bass_kernel_spmd(_get_nc(), in_maps, list(range(N_CORES)))
    return np.ascontiguousarray(
        np.concatenate([r["out"] for r in res.results], axis=0).astype(np.float32)
    )



# revision 3
# speedup vs baseline: 1.1764x; 1.1764x over previous
"""Compressed Interaction Network (CIN) forward on 8 Trainium2 NeuronCores.

Math (per batch item, m=32 fields, d=64 embed, H=256 hidden):
    x0 = x[i]                          # (m, d)
    h  = x0
    layer l in 0..2:
        z = outer(x0, h) over d        # (m*n, d), z[(a,b),:] = x0[a,:]*h[b,:]
        y = relu(W_l^T z + b_l)        # (H, d)
        xcur, h = split_half(y) (layers 0,1); xcur = h = y (layer 2)
    f = concat(xcur_0, xcur_1, xcur_2) # (512, d)
    out[i] = sum_d(f) @ fc_W + fc_b    # scalar

Mapping: batch 1024 -> 8 cores x 128 items, 16 groups of 8 items per core.
 - Outer-product operands built with DMA broadcast reads (stride-0 APs).
 - Layer-0 z uses a host-retiled xc tensor so each z chunk is ONE
   128-partition vector op (the per-partition field index p//32 is baked
   into xc's layout); layers 1/2 broadcast h along the 4-field axis.
 - Conv matmuls on PE: stationary W chunks [128, 128] fp16, moving z
   [128, 512] (8 items x 64 d), accumulated over k-chunks in fp32 PSUM.
 - Bias+ReLU fused into the PSUM->SBUF move on ScalarE; the relu'd xcur
   chunks go back through the PE against fc_W chunks, accumulating the
   per-(item,d) FC partial dot in a [1, 512] PSUM bank; one small
   VectorE reduce per group finishes the sum over d.
"""

import numpy as np

import concourse.bass as bass
import concourse.tile as tile
from concourse import mybir
from concourse.bass_utils import run_bass_kernel_spmd

N_CORES = 8
B_TOTAL = 1024
B_CORE = B_TOTAL // N_CORES  # 128
M = 32  # num fields
D = 64  # embed dim
H = 256  # conv output channels
GROUP = 8  # items per group (512 moving columns)
N_GROUPS = B_CORE // GROUP  # 16
MD = M * D  # 2048, elements per item row

F16 = mybir.dt.float16
F32 = mybir.dt.float32
RELU = mybir.ActivationFunctionType.Relu
IDENT = mybir.ActivationFunctionType.Identity
AXX = mybir.AxisListType.X
ADD = mybir.AluOpType.add


def build():
    nc = bass.Bass()
    xh = nc.declare_dram_parameter("xh", [B_CORE, M, D], F16, isOutput=False)
    # x rows tiled 4x along the field axis: xr[i, p, d] = x[i, p % 32, d]
    xr = nc.declare_dram_parameter("xr", [B_CORE, 128, D], F16, isOutput=False)
    # layer-0 z operand: xc[i, p, c, d] = x[i, 4c + p//32, d]
    xc = nc.declare_dram_parameter("xc", [B_CORE, 128, 8, D], F16, isOutput=False)
    w0 = nc.declare_dram_parameter("w0", [8, 128, H], F16, isOutput=False)
    w1 = nc.declare_dram_parameter("w1", [32, 128, H], F16, isOutput=False)
    w2 = nc.declare_dram_parameter("w2", [32, 128, H], F16, isOutput=False)
    bia = nc.declare_dram_parameter("bia", [128, 3, 2], F32, isOutput=False)
    fcw = nc.declare_dram_parameter("fcw", [128, 4], F16, isOutput=False)
    fcb = nc.declare_dram_parameter("fcb", [1, 1], F32, isOutput=False)
    out = nc.declare_dram_parameter("out", [B_CORE, 1], F32, isOutput=True)

    with tile.TileContext(nc) as tc:
        with (
            tc.tile_pool(name="consts", bufs=1) as consts,
            tc.tile_pool(name="bpool", bufs=2) as bpool,
            tc.tile_pool(name="cpool", bufs=2) as cpool,
            tc.tile_pool(name="epool", bufs=2) as epool,
            tc.tile_pool(name="z0pool", bufs=4) as z0pool,
            tc.tile_pool(name="zpool", bufs=3) as zpool,
            tc.tile_pool(name="hpool", bufs=4) as hpool,
            tc.tile_pool(name="rxpool", bufs=6) as rxpool,
            tc.tile_pool(name="ppool", bufs=6, space="PSUM") as ppool,
            tc.tile_pool(name="fcp", bufs=2, space="PSUM") as fcp,
        ):
            # --- resident constants ---
            w0_sb = consts.tile([128, 8, H], F16, tag="w0")
            nc.sync.dma_start(w0_sb[:], w0[:].rearrange("c k o -> k c o"))
            w1_sb = consts.tile([128, 32, H], F16, tag="w1")
            nc.sync.dma_start(w1_sb[:], w1[:].rearrange("c k o -> k c o"))
            w2_sb = consts.tile([128, 32, H], F16, tag="w2")
            nc.sync.dma_start(w2_sb[:], w2[:].rearrange("c k o -> k c o"))
            bia_sb = consts.tile([128, 3, 2], F32, tag="bia")
            nc.sync.dma_start(bia_sb[:], bia[:])
            fcw_sb = consts.tile([128, 4], F16, tag="fcw")
            nc.sync.dma_start(fcw_sb[:], fcw[:])
            fcb_sb = consts.tile([1, 1], F32, tag="fcb")
            nc.sync.dma_start(fcb_sb[:], fcb[:])

            # per-item FC dot results, [1, item]
            osb = consts.tile([1, B_CORE], F32, tag="osb")

            for g in range(N_GROUPS):
                i0 = g * GROUP

                # B[p, i, m, d] = x_i[m, d] for every partition p
                Bg = bpool.tile([128, GROUP, M, D], F16, tag="B")
                src = bass.AP(
                    tensor=xh,
                    offset=i0 * MD,
                    ap=[[0, 128], [MD, GROUP], [1, MD]],
                )
                nc.sync.dma_start(Bg[:], src)

                # R[p, i, d] = x_i[p % 32, d]  (host-tiled)
                Rg = epool.tile([128, GROUP, D], F16, tag="R")
                src = bass.AP(
                    tensor=xr,
                    offset=i0 * 128 * D,
                    ap=[[D, 128], [128 * D, GROUP], [1, D]],
                )
                nc.sync.dma_start(Rg[:], src)

                # C[p, i, c, d] = x_i[4c + p//32, d]  (host-tiled)
                Cg = cpool.tile([128, GROUP, 8, D], F16, tag="C")
                src = bass.AP(
                    tensor=xc,
                    offset=i0 * 128 * 8 * D,
                    ap=[[8 * D, 128], [128 * 8 * D, GROUP], [1, 8 * D]],
                )
                nc.sync.dma_start(Cg[:], src)

                # ---------- layer 0 ----------
                # chunk c of k: z0[p] = x[p%32] * x[4c + p//32], one op each
                ps0 = [
                    ppool.tile([128, GROUP * D], F32, tag="yps", name="ps0")
                    for _ in range(2)
                ]
                for c in range(8):
                    z0 = z0pool.tile([128, GROUP, D], F16, tag="z0")
                    nc.vector.tensor_mul(z0[:], Rg[:], Cg[:, :, c, :])
                    for oc in range(2):
                        nc.tensor.matmul(
                            ps0[oc][:],
                            w0_sb[:, c, oc * 128 : (oc + 1) * 128],
                            z0[:],
                            start=(c == 0),
                            stop=(c == 7),
                        )

                # psum -> sbuf with fused bias+relu; chunk1 becomes next h,
                # chunk0 (xcur) is relu'd to SBUF for the FC partial dot
                h1 = hpool.tile([128, GROUP, D], F16, tag="h1")
                nc.scalar.activation(h1[:], ps0[1][:], RELU, bias=bia_sb[:, 0, 1:2])
                rx0 = rxpool.tile([128, GROUP * D], F16, tag="rx")
                nc.scalar.activation(rx0[:], ps0[0][:], RELU, bias=bia_sb[:, 0, 0:1])

                fc_ps = fcp.tile([1, GROUP * D], F32, tag="fc")

                # ---------- layers 1 and 2 ----------
                for lay in range(2):
                    w_sb = w1_sb if lay == 0 else w2_sb
                    h_in = h1 if lay == 0 else h2
                    ps = [
                        ppool.tile([128, GROUP * D], F32, tag="yps", name="ps")
                        for _ in range(2)
                    ]
                    for mb in range(8):
                        zt = zpool.tile([128, GROUP, 4, D], F16, tag="z")
                        nc.vector.tensor_mul(
                            zt[:],
                            h_in[:, :, None, :].to_broadcast((128, GROUP, 4, D)),
                            Bg[:, :, 4 * mb : 4 * mb + 4, :],
                        )
                        for mm in range(4):
                            m = 4 * mb + mm
                            for oc in range(2):
                                nc.tensor.matmul(
                                    ps[oc][:],
                                    w_sb[:, m, oc * 128 : (oc + 1) * 128],
                                    zt[:, :, mm, :],
                                    start=(m == 0),
                                    stop=(m == 31),
                                )
                        if lay == 0 and mb == 0:
                            # slot the L0 fc matmul behind L1's first MMs so
                            # ScalarE has time to produce rx0
                            nc.tensor.matmul(
                                fc_ps[:],
                                fcw_sb[:, 0:1],
                                rx0[:],
                                start=True,
                                stop=False,
                            )
                    if lay == 0:
                        h2 = hpool.tile([128, GROUP, D], F16, tag="h2")
                        nc.scalar.activation(
                            h2[:], ps[1][:], RELU, bias=bia_sb[:, 1, 1:2]
                        )
                        rx1 = rxpool.tile([128, GROUP * D], F16, tag="rx")
                        nc.scalar.activation(
                            rx1[:], ps[0][:], RELU, bias=bia_sb[:, 1, 0:1]
                        )
                    else:
                        rx2 = [
                            rxpool.tile([128, GROUP * D], F16, tag="rx", name="rx2")
                            for _ in range(2)
                        ]
                        for oc in range(2):
                            nc.scalar.activation(
                                rx2[oc][:],
                                ps[oc][:],
                                RELU,
                                bias=bia_sb[:, 2, oc : oc + 1],
                            )

                # remaining FC partial dots (chunk1 slotted here so rx1 is
                # ready; chunks 2,3 at group end)
                nc.tensor.matmul(
                    fc_ps[:], fcw_sb[:, 1:2], rx1[:], start=False, stop=False
                )
                for oc in range(2):
                    nc.tensor.matmul(
                        fc_ps[:],
                        fcw_sb[:, 2 + oc : 3 + oc],
                        rx2[oc][:],
                        start=False,
                        stop=(oc == 1),
                    )
                # sum over d: [1, (i, d)] -> [1, i]
                nc.vector.tensor_reduce(
                    osb[0:1, i0 : i0 + GROUP],
                    fc_ps[:].rearrange("p (i d) -> p i d", i=GROUP),
                    axis=AXX,
                    op=ADD,
                )

            # ---------- finalize: add fc bias, write out ----------
            osb2 = consts.tile([1, B_CORE], F32, tag="osb2")
            nc.scalar.activation(osb2[:], osb[:], IDENT, bias=fcb_sb[0:1, 0:1])
            nc.sync.dma_start(out[:], osb2[:])

    _legalize_waits(nc)
    return nc


def _legalize_waits(nc, max_waits=1):
    """walrus codegen allows at most 2 semaphore waits per instruction; spill
    the excess onto NoOps injected just before the offender on the same
    engine (same-engine FIFO makes this ordering-equivalent)."""
    for bb in nc.main_func.blocks:
        insts = bb.instructions
        new_list = []
        changed = False
        for ins in insts:
            si = ins.sync_info
            if si is not None and si.on_wait and len(si.on_wait) > max_waits:
                waits = list(si.on_wait)
                extra, keep = waits[:-max_waits], waits[-max_waits:]
                k = 0
                while k < len(extra):
                    chunk = extra[k : k + max_waits]
                    nop = mybir.InstNoOp(name=f"{ins.name}-w{k}", ins=[], outs=[])
                    nop.engine = ins.engine
                    nop.sync_info = mybir.SyncInfo(on_wait=chunk, on_update=[])
                    new_list.append(nop)
                    k += max_waits
                ins.sync_info = mybir.SyncInfo(
                    on_wait=keep,
                    on_update=list(si.on_update) if si.on_update else [],
                )
                changed = True
            new_list.append(ins)
        if changed:
            if hasattr(bb, "set_instructions"):
                bb.set_instructions(new_list)
            else:
                insts.clear()
                insts.extend(new_list)
                if len(bb.instructions) != len(new_list):
                    bb.instructions = new_list


def prep_inputs(x, W0, b0, W1, b1, W2, b2, fc_W, fc_b):
    """Host-side reshape/cast into the per-core input maps."""
    xh = np.ascontiguousarray(x.astype(np.float16))
    xr = np.ascontiguousarray(
        np.tile(xh.reshape(B_TOTAL, 1, M, D), (1, 4, 1, 1)).reshape(
            B_TOTAL, 128, D
        )
    )
    # xc[i, p, c, d] = x[i, 4c + p//32, d]
    idx = (4 * np.arange(8)[None, :] + (np.arange(128) // 32)[:, None])  # (128, 8)
    xc = np.ascontiguousarray(xh[:, idx, :])  # (B, 128, 8, D)
    w0 = np.ascontiguousarray(W0.astype(np.float16).reshape(8, 128, H))
    w1 = np.ascontiguousarray(W1.astype(np.float16).reshape(32, 128, H))
    w2 = np.ascontiguousarray(W2.astype(np.float16).reshape(32, 128, H))
    bia = np.ascontiguousarray(
        np.stack([b0, b1, b2]).reshape(3, 2, 128).transpose(2, 0, 1).astype(np.float32)
    )
    fcw = np.ascontiguousarray(fc_W.reshape(4, 128).T.astype(np.float16))
    fcb = np.ascontiguousarray(fc_b.reshape(1, 1).astype(np.float32))
    shared = {"w0": w0, "w1": w1, "w2": w2, "bia": bia, "fcw": fcw, "fcb": fcb}
    return [
        {
            "xh": xh[i * B_CORE : (i + 1) * B_CORE],
            "xr": xr[i * B_CORE : (i + 1) * B_CORE],
            "xc": xc[i * B_CORE : (i + 1) * B_CORE],
            **shared,
        }
        for i in range(N_CORES)
    ]


_NC = None


def _get_nc():
    global _NC
    if _NC is None:
        _NC = build()
    return _NC


def kernel(**inputs):
    in_maps = prep_inputs(**inputs)
    res = run_bass_kernel_spmd(_get_nc(), in_maps, list(range(N_CORES)))
    return np.ascontiguousarray(
        np.concatenate([r["out"] for r in res.results], axis=0).astype(np.float32)
    )


# revision 5
# speedup vs baseline: 1.3621x; 1.1579x over previous
"""Compressed Interaction Network (CIN) forward on 8 Trainium2 NeuronCores.

Math (per batch item, m=32 fields, d=64 embed, H=256 hidden):
    x0 = x[i]                          # (m, d)
    h  = x0
    layer l in 0..2:
        z = outer(x0, h) over d        # (m*n, d), z[(a,b),:] = x0[a,:]*h[b,:]
        y = relu(W_l^T z + b_l)        # (H, d)
        xcur, h = split_half(y) (layers 0,1); xcur = h = y (layer 2)
    f = concat(xcur_0, xcur_1, xcur_2) # (512, d)
    out[i] = sum_d(f) @ fc_W + fc_b    # scalar

Mapping: batch 1024 -> 8 cores x 128 items, 16 groups of 8 items per core.

The three layers of a group are software-pipelined across emission rounds so
the PE never stalls on the ScalarE h-drain -> VectorE z-production latency:
round r runs L0(r), L1(r-1), L2(r-2), fc(r-3) back to back on the PE while
VectorE builds the z tiles one round ahead of their consumption.
 - z operands via DMA broadcast reads; layer-0's z is ONE 128-partition
   vector op per k-chunk thanks to a host-retiled xc tensor (the
   per-partition field index p//32 is baked into xc's layout).
 - Conv matmuls on PE: stationary W chunks [128, 128] fp16, moving z
   [128, 512] (8 items x 64 d), accumulated over k-chunks in fp32 PSUM.
 - Bias+ReLU fused into the PSUM->SBUF drain on ScalarE; the relu'd xcur
   chunks go back through the PE against fc_W chunks, accumulating the
   per-(item,d) FC partial dot in a [1, 512] PSUM bank; one small VectorE
   reduce per group finishes the sum over d.
"""

import numpy as np

import concourse.bass as bass
import concourse.tile as tile
from concourse import mybir
from concourse.bass_utils import run_bass_kernel_spmd

N_CORES = 8
B_TOTAL = 1024
B_CORE = B_TOTAL // N_CORES  # 128
M = 32  # num fields
D = 64  # embed dim
H = 256  # conv output channels
GROUP = 8  # items per group (512 moving columns)
NG = B_CORE // GROUP  # 16 groups
MD = M * D  # 2048, elements per item row

F16 = mybir.dt.float16
F32 = mybir.dt.float32
RELU = mybir.ActivationFunctionType.Relu
IDENT = mybir.ActivationFunctionType.Identity
AXX = mybir.AxisListType.X
ADD = mybir.AluOpType.add


def build():
    nc = bass.Bass()
    xh = nc.declare_dram_parameter("xh", [B_CORE, M, D], F16, isOutput=False)
    # x rows tiled 4x along the field axis: xr[i, p, d] = x[i, p % 32, d]
    xr = nc.declare_dram_parameter("xr", [B_CORE, 128, D], F16, isOutput=False)
    # layer-0 z operand: xc[i, p, c, d] = x[i, 4c + p//32, d]
    xc = nc.declare_dram_parameter("xc", [B_CORE, 128, 8, D], F16, isOutput=False)
    w0 = nc.declare_dram_parameter("w0", [8, 128, H], F16, isOutput=False)
    w1 = nc.declare_dram_parameter("w1", [32, 128, H], F16, isOutput=False)
    w2 = nc.declare_dram_parameter("w2", [32, 128, H], F16, isOutput=False)
    bia = nc.declare_dram_parameter("bia", [128, 3, 2], F32, isOutput=False)
    fcw = nc.declare_dram_parameter("fcw", [128, 4], F16, isOutput=False)
    fcb = nc.declare_dram_parameter("fcb", [1, 1], F32, isOutput=False)
    out = nc.declare_dram_parameter("out", [B_CORE, 1], F32, isOutput=True)

    with tile.TileContext(nc) as tc:
        with (
            tc.tile_pool(name="consts", bufs=1) as consts,
            tc.tile_pool(name="bpool", bufs=3) as bpool,
            tc.tile_pool(name="cpool", bufs=2) as cpool,
            tc.tile_pool(name="epool", bufs=2) as epool,
            tc.tile_pool(name="z0pool", bufs=10) as z0pool,
            tc.tile_pool(name="ztpool", bufs=3) as ztpool,
            tc.tile_pool(name="hpool", bufs=3) as hpool,
            tc.tile_pool(name="rxpool", bufs=5) as rxpool,
            tc.tile_pool(name="ppool", bufs=6, space="PSUM") as ppool,
            tc.tile_pool(name="fcp", bufs=2, space="PSUM") as fcp,
        ):
            # --- constants; w1/w2 DMAs are deferred into round 0 so they
            # don't delay group 0's data on the SP queue ---
            w0_sb = consts.tile([128, 8, H], F16, tag="w0")
            nc.sync.dma_start(w0_sb[:], w0[:].rearrange("c k o -> k c o"))
            bia_sb = consts.tile([128, 3, 2], F32, tag="bia")
            nc.sync.dma_start(bia_sb[:], bia[:])
            fcw_sb = consts.tile([128, 4], F16, tag="fcw")
            nc.sync.dma_start(fcw_sb[:], fcw[:])
            fcb_sb = consts.tile([1, 1], F32, tag="fcb")
            nc.sync.dma_start(fcb_sb[:], fcb[:])
            w1_sb = consts.tile([128, 32, H], F16, tag="w1")
            w2_sb = consts.tile([128, 32, H], F16, tag="w2")

            # per-item FC dot results, [1, item]
            osb = consts.tile([1, B_CORE], F32, tag="osb")

            # pipeline state carried between rounds
            Cg = {}
            Rg = {}
            Bg = {}
            z0t = {}
            h1t = {}
            h2t = {}
            rx0t = {}
            rx1t = {}
            rx2t = {}
            ps0t = {}
            ps1t = {}
            ps2t = {}

            def dma_group_cr(g):
                Cg[g] = cpool.tile([128, GROUP, 8, D], F16, tag="C", name="Cg")
                src = bass.AP(
                    tensor=xc,
                    offset=g * GROUP * 128 * 8 * D,
                    ap=[[8 * D, 128], [128 * 8 * D, GROUP], [1, 8 * D]],
                )
                nc.sync.dma_start(Cg[g][:], src)
                Rg[g] = epool.tile([128, GROUP, D], F16, tag="R", name="Rg")
                src = bass.AP(
                    tensor=xr,
                    offset=g * GROUP * 128 * D,
                    ap=[[D, 128], [128 * D, GROUP], [1, D]],
                )
                nc.sync.dma_start(Rg[g][:], src)

            def dma_group_b(g):
                Bg[g] = bpool.tile([128, GROUP, M, D], F16, tag="B", name="Bg")
                src = bass.AP(
                    tensor=xh,
                    offset=g * GROUP * MD,
                    ap=[[0, 128], [MD, GROUP], [1, MD]],
                )
                nc.sync.dma_start(Bg[g][:], src)

            def vec_z0(g):
                z0t[g] = [
                    z0pool.tile([128, GROUP, D], F16, tag="z0", name="z0")
                    for _ in range(8)
                ]
                for c in range(8):
                    nc.vector.tensor_mul(z0t[g][c][:], Rg[g][:], Cg[g][:, :, c, :])

            # ---------- preamble: group 0's z0 ----------
            dma_group_cr(0)
            vec_z0(0)

            for r in range(NG + 3):
                # 1. prefetch DMAs
                if r + 1 < NG:
                    dma_group_cr(r + 1)
                if r < NG:
                    dma_group_b(r)
                if r == 0:
                    nc.sync.dma_start(w1_sb[:], w1[:].rearrange("c k o -> k c o"))
                    nc.sync.dma_start(w2_sb[:], w2[:].rearrange("c k o -> k c o"))

                # 2. vector: zt for L1(r-1)
                if 0 <= r - 1 < NG:
                    g = r - 1
                    zl1 = [
                        ztpool.tile([128, GROUP, 4, D], F16, tag="z1", name="zl1")
                        for _ in range(8)
                    ]
                    for mb in range(8):
                        nc.vector.tensor_mul(
                            zl1[mb][:],
                            h1t[g][:, :, None, :].to_broadcast((128, GROUP, 4, D)),
                            Bg[g][:, :, 4 * mb : 4 * mb + 4, :],
                        )

                # 3. PE: L0(r)
                if r < NG:
                    ps0t[r] = [
                        ppool.tile([128, GROUP * D], F32, tag="yps", name="ps0")
                        for _ in range(2)
                    ]
                    for c in range(8):
                        for oc in range(2):
                            nc.tensor.matmul(
                                ps0t[r][oc][:],
                                w0_sb[:, c, oc * 128 : (oc + 1) * 128],
                                z0t[r][c][:],
                                start=(c == 0),
                                stop=(c == 7),
                            )
                    del z0t[r]

                # 4. scalar: drain L0(r)
                if r < NG:
                    h1t[r] = hpool.tile([128, GROUP, D], F16, tag="h1", name="h1")
                    nc.scalar.activation(
                        h1t[r][:], ps0t[r][1][:], RELU, bias=bia_sb[:, 0, 1:2]
                    )
                    rx0t[r] = rxpool.tile([128, GROUP * D], F16, tag="rx0", name="rx0")
                    nc.scalar.activation(
                        rx0t[r][:], ps0t[r][0][:], RELU, bias=bia_sb[:, 0, 0:1]
                    )
                    del ps0t[r]

                # 5. PE: L1(r-1)
                if 0 <= r - 1 < NG:
                    g = r - 1
                    ps1t[g] = [
                        ppool.tile([128, GROUP * D], F32, tag="yps", name="ps1")
                        for _ in range(2)
                    ]
                    for mb in range(8):
                        for mm in range(4):
                            m = 4 * mb + mm
                            for oc in range(2):
                                nc.tensor.matmul(
                                    ps1t[g][oc][:],
                                    w1_sb[:, m, oc * 128 : (oc + 1) * 128],
                                    zl1[mb][:, :, mm, :],
                                    start=(m == 0),
                                    stop=(m == 31),
                                )

                # 6. vector: z0(r+1)
                if r + 1 < NG:
                    vec_z0(r + 1)

                # 7. scalar: drain L1(r-1)
                if 0 <= r - 1 < NG:
                    g = r - 1
                    h2t[g] = hpool.tile([128, GROUP, D], F16, tag="h2", name="h2")
                    nc.scalar.activation(
                        h2t[g][:], ps1t[g][1][:], RELU, bias=bia_sb[:, 1, 1:2]
                    )
                    rx1t[g] = rxpool.tile([128, GROUP * D], F16, tag="rx1", name="rx1")
                    nc.scalar.activation(
                        rx1t[g][:], ps1t[g][0][:], RELU, bias=bia_sb[:, 1, 0:1]
                    )
                    del ps1t[g], h1t[g]

                # 8. vector: zt for L2(r-2)
                if 0 <= r - 2 < NG:
                    g = r - 2
                    zl2 = [
                        ztpool.tile([128, GROUP, 4, D], F16, tag="z2", name="zl2")
                        for _ in range(8)
                    ]
                    for mb in range(8):
                        nc.vector.tensor_mul(
                            zl2[mb][:],
                            h2t[g][:, :, None, :].to_broadcast((128, GROUP, 4, D)),
                            Bg[g][:, :, 4 * mb : 4 * mb + 4, :],
                        )

                # 9. PE: L2(r-2)
                if 0 <= r - 2 < NG:
                    g = r - 2
                    ps2t[g] = [
                        ppool.tile([128, GROUP * D], F32, tag="yps", name="ps2")
                        for _ in range(2)
                    ]
                    for mb in range(8):
                        for mm in range(4):
                            m = 4 * mb + mm
                            for oc in range(2):
                                nc.tensor.matmul(
                                    ps2t[g][oc][:],
                                    w2_sb[:, m, oc * 128 : (oc + 1) * 128],
                                    zl2[mb][:, :, mm, :],
                                    start=(m == 0),
                                    stop=(m == 31),
                                )

                # 10. scalar: drain L2(r-2)
                if 0 <= r - 2 < NG:
                    g = r - 2
                    rx2t[g] = [
                        rxpool.tile([128, GROUP * D], F16, tag="rx2", name="rx2")
                        for _ in range(2)
                    ]
                    for oc in range(2):
                        nc.scalar.activation(
                            rx2t[g][oc][:],
                            ps2t[g][oc][:],
                            RELU,
                            bias=bia_sb[:, 2, oc : oc + 1],
                        )
                    del ps2t[g], h2t[g], Bg[g]

                # 11. PE + vector: FC dot for group r-3 (all rx ready)
                if 0 <= r - 3 < NG:
                    g = r - 3
                    fc_ps = fcp.tile([1, GROUP * D], F32, tag="fc", name="fc")
                    rxs = [rx0t[g], rx1t[g], rx2t[g][0], rx2t[g][1]]
                    for c in range(4):
                        nc.tensor.matmul(
                            fc_ps[:],
                            fcw_sb[:, c : c + 1],
                            rxs[c][:],
                            start=(c == 0),
                            stop=(c == 3),
                        )
                    nc.vector.tensor_reduce(
                        osb[0:1, g * GROUP : (g + 1) * GROUP],
                        fc_ps[:].rearrange("p (i d) -> p i d", i=GROUP),
                        axis=AXX,
                        op=ADD,
                    )
                    del rx0t[g], rx1t[g], rx2t[g]

            # ---------- finalize: add fc bias, write out ----------
            osb2 = consts.tile([1, B_CORE], F32, tag="osb2")
            nc.scalar.activation(osb2[:], osb[:], IDENT, bias=fcb_sb[0:1, 0:1])
            nc.sync.dma_start(out[:], osb2[:])

    _legalize_waits(nc)
    return nc


def _legalize_waits(nc, max_waits=1):
    """walrus codegen allows at most 2 semaphore waits per instruction; spill
    the excess onto NoOps injected just before the offender on the same
    engine (same-engine FIFO makes this ordering-equivalent)."""
    for bb in nc.main_func.blocks:
        insts = bb.instructions
        new_list = []
        changed = False
        for ins in insts:
            si = ins.sync_info
            if si is not None and si.on_wait and len(si.on_wait) > max_waits:
                waits = list(si.on_wait)
                extra, keep = waits[:-max_waits], waits[-max_waits:]
                k = 0
                while k < len(extra):
                    chunk = extra[k : k + max_waits]
                    nop = mybir.InstNoOp(name=f"{ins.name}-w{k}", ins=[], outs=[])
                    nop.engine = ins.engine
                    nop.sync_info = mybir.SyncInfo(on_wait=chunk, on_update=[])
                    new_list.append(nop)
                    k += max_waits
                ins.sync_info = mybir.SyncInfo(
                    on_wait=keep,
                    on_update=list(si.on_update) if si.on_update else [],
                )
                changed = True
            new_list.append(ins)
        if changed:
            if hasattr(bb, "set_instructions"):
                bb.set_instructions(new_list)
            else:
                insts.clear()
                insts.extend(new_list)
                if len(bb.instructions) != len(new_list):
                    bb.instructions = new_list


def prep_inputs(x, W0, b0, W1, b1, W2, b2, fc_W, fc_b):
    """Host-side reshape/cast into the per-core input maps."""
    xh = np.ascontiguousarray(x.astype(np.float16))
    xr = np.ascontiguousarray(
        np.tile(xh.reshape(B_TOTAL, 1, M, D), (1, 4, 1, 1)).reshape(
            B_TOTAL, 128, D
        )
    )
    # xc[i, p, c, d] = x[i, 4c + p//32, d]
    idx = (4 * np.arange(8)[None, :] + (np.arange(128) // 32)[:, None])  # (128, 8)
    xc = np.ascontiguousarray(xh[:, idx, :])  # (B, 128, 8, D)
    w0 = np.ascontiguousarray(W0.astype(np.float16).reshape(8, 128, H))
    w1 = np.ascontiguousarray(W1.astype(np.float16).reshape(32, 128, H))
    w2 = np.ascontiguousarray(W2.astype(np.float16).reshape(32, 128, H))
    bia = np.ascontiguousarray(
        np.stack([b0, b1, b2]).reshape(3, 2, 128).transpose(2, 0, 1).astype(np.float32)
    )
    fcw = np.ascontiguousarray(fc_W.reshape(4, 128).T.astype(np.float16))
    fcb = np.ascontiguousarray(fc_b.reshape(1, 1).astype(np.float32))
    shared = {"w0": w0, "w1": w1, "w2": w2, "bia": bia, "fcw": fcw, "fcb": fcb}
    return [
        {
            "xh": xh[i * B_CORE : (i + 1) * B_CORE],
            "xr": xr[i * B_CORE : (i + 1) * B_CORE],
            "xc": xc[i * B_CORE : (i + 1) * B_CORE],
            **shared,
        }
        for i in range(N_CORES)
    ]


_NC = None


def _get_nc():
    global _NC
    if _NC is None:
        _NC = build()
    return _NC


def kernel(**inputs):
    in_maps = prep_inputs(**inputs)
    res = run_bass_kernel_spmd(_get_nc(), in_maps, list(range(N_CORES)))
    return np.ascontiguousarray(
        np.concatenate([r["out"] for r in res.results], axis=0).astype(np.float32)
    )
